# revision 8
# baseline (speedup 1.0000x reference)
"""MMD (Maximum Mean Discrepancy) loss kernel for Trainium2, 8 NeuronCores.

Math: with x = concat(source, target) [N=8192, D=256],
  L2_ij = sq_i + sq_j - 2 x_i.x_j
  bandwidth = sum(L2) / (N^2-N) / 4        (closed form: sum(L2) = 2N*sum(sq) - 2||colsum(x)||^2)
  K = sum_b exp(-L2 / (bandwidth * 2^b)), b=0..4
  loss = mean(K_SS) + mean(K_TT) - 2*mean(K_ST)

Device strategy (row-sharded, core m owns rows [1024m, 1024(m+1))):
  - One matmul group per G-tile computes G''_ij = x_i.x_j + (c - 0.5 sq_j) via
    an augmented contraction row (K = 256 + 1), in float32r (full PE rate).
  - ScalarE activation computes exp(scale_b * G'' + bias_b[i]) with
    scale_b = 2*a_b (per-partition AP) and bias_b[i] = -a_b*sq_i - 2*a_b*c,
    which equals exp(-a_b * L2_ij) exactly; the fused accum_out reduces
    along the free dim, so no DVE work in the hot path.
  - Per-core output is a [128, 8*4*5] grid of partial row-sums (iblk x
    2048-col group x bandwidth); host applies block signs and divides by B^2.

Constraint baked into the DMA issue order: an fp32r Matmult can carry only ONE
sync-wait (walrus "Too many sync wait commands" otherwise). HWDGE DMAs are
round-robined over 8 queues in issue order, so the weights DMA is issued 9th
to land on queue 0 together with the first rhs chunk (one semaphore covers
both), and ones/scale ride inside the aug/bias tensors respectively.
"""

import numpy as np

B = 4096
D = 256
N = 2 * B
KERNEL_MUL = 2.0
KERNEL_NUM = 5
NCORES = 8
ROWS_PER_CORE = N // NCORES  # 1024
NIBLK = ROWS_PER_CORE // 128  # 8
JG = 2048  # columns per ACT accumulation chunk (4 PSUM banks)
NJG = N // JG  # 4
JT = 512  # columns per matmul (1 PSUM bank)
NJT = JG // JT  # 4

_CACHE = {}


def _build_program():
    import concourse.bass as bass
    import concourse.tile as tile
    from concourse import bacc, mybir

    f32 = mybir.dt.float32
    f32r = mybir.dt.float32r
    Exp = mybir.ActivationFunctionType.Exp

    nc = bacc.Bacc(None)

    xT = nc.declare_dram_parameter("xT", [128, NJG, 2, JG], f32r, isOutput=False)
    wT = nc.declare_dram_parameter("wT", [128, 2, ROWS_PER_CORE], f32r, isOutput=False)
    # aug2 cols [0,N): c - 0.5*sq_j ; cols [N, N+128): ones (the K=1 lhsT)
    aug = nc.declare_dram_parameter("aug2", [1, N + 128], f32r, isOutput=False)
    # biasscale cols [0, NIBLK*5): bias ; cols [NIBLK*5, NIBLK*5+5): scale
    bsc = nc.declare_dram_parameter(
        "biasscale", [128, NIBLK * KERNEL_NUM + KERNEL_NUM], f32, isOutput=False
    )
    res = nc.declare_dram_parameter(
        "res", [128, NIBLK * NJG * KERNEL_NUM], f32, isOutput=True
    )

    with tile.TileContext(nc) as tc:
        with (
            tc.tile_pool(name="sing", bufs=1) as sing,
            tc.tile_pool(name="scr", bufs=2) as scr,
            tc.tile_pool(name="psum", bufs=2, space=bass.MemorySpace.PSUM) as psum,
        ):
            rhs_sb = sing.tile([128, NJG, 2, JG], f32r)
            w_sb = sing.tile([128, 2, ROWS_PER_CORE], f32r)
            aug_sb = sing.tile([1, N + 128], f32r)
            bsc_sb = sing.tile([128, NIBLK * KERNEL_NUM + KERNEL_NUM], f32)
            res_sb = sing.tile([128, NIBLK * NJG * KERNEL_NUM], f32)

            # HWDGE queue assignment is round-robin in issue order (8 queues).
            # Keep this order: rhs jg0 lands on queue 0 and wT (9th) also on
            # queue 0 so the first matmul needs a single semaphore wait.
            nc.sync.dma_start(out=rhs_sb[:, 0], in_=xT[:, 0])  # q0
            nc.sync.dma_start(out=bsc_sb, in_=bsc[:])  # q1
            nc.sync.dma_start(out=aug_sb, in_=aug[:])  # q2
            nc.sync.dma_start(out=rhs_sb[:, 1, 0], in_=xT[:, 1, 0])  # q3
            nc.sync.dma_start(out=rhs_sb[:, 1, 1], in_=xT[:, 1, 1])  # q4
            nc.sync.dma_start(out=rhs_sb[:, 2, 0], in_=xT[:, 2, 0])  # q5
            nc.sync.dma_start(out=rhs_sb[:, 2, 1], in_=xT[:, 2, 1])  # q6
            nc.sync.dma_start(out=rhs_sb[:, 3], in_=xT[:, 3])  # q7
            nc.sync.dma_start(out=w_sb, in_=wT[:])  # q0 again

            for iblk in range(NIBLK):
                for jg in range(NJG):
                    pt = psum.tile([128, JG], f32)
                    for jt in range(NJT):
                        j0 = jg * JG + jt * JT
                        out_sl = pt[:, jt * JT : (jt + 1) * JT]
                        nc.tensor.matmul(
                            out_sl,
                            w_sb[:, 0, iblk * 128 : (iblk + 1) * 128],
                            rhs_sb[:, jg, 0, jt * JT : (jt + 1) * JT],
                            start=True,
                            stop=False,
                        )
                        nc.tensor.matmul(
                            out_sl,
                            w_sb[:, 1, iblk * 128 : (iblk + 1) * 128],
                            rhs_sb[:, jg, 1, jt * JT : (jt + 1) * JT],
                            start=False,
                            stop=False,
                        )
                        nc.tensor.matmul(
                            out_sl,
                            aug_sb[0:1, N : N + 128],
                            aug_sb[0:1, j0 : j0 + JT],
                            start=False,
                            stop=True,
                        )
                    for b in range(KERNEL_NUM):
                        sc = scr.tile([128, JG], f32)
                        idx = (iblk * NJG + jg) * KERNEL_NUM + b
                        bidx = iblk * KERNEL_NUM + b
                        nc.scalar.activation(
                            out=sc[:],
                            in_=pt[:],
                            func=Exp,
                            bias=bsc_sb[:, bidx : bidx + 1],
                            scale=bsc_sb[
                                :,
                                NIBLK * KERNEL_NUM + b : NIBLK * KERNEL_NUM + b + 1,
                            ],
                            accum_out=res_sb[:, idx : idx + 1],
                        )

            nc.sync.dma_start(out=res[:], in_=res_sb[:])

    nc.finalize()
    return nc


def _get_program():
    if "nc" not in _CACHE:
        _CACHE["nc"] = _build_program()
    return _CACHE["nc"]


def _host_prep(source_features, target_features):
    """Build per-core input maps. All heavy data stays fp32; scalars in fp64."""
    x = np.concatenate(
        [np.asarray(source_features, np.float32), np.asarray(target_features, np.float32)],
        axis=0,
    )  # [N, D]
    x64 = x.astype(np.float64)
    sq = np.sum(x64 * x64, axis=1)  # [N]
    colsum = np.sum(x64, axis=0)  # [D]
    sum_l2 = 2.0 * N * np.sum(sq) - 2.0 * np.dot(colsum, colsum)
    bandwidth = sum_l2 / (N * N - N) / (KERNEL_MUL ** (KERNEL_NUM // 2))
    a = np.array([1.0 / (bandwidth * KERNEL_MUL**b) for b in range(KERNEL_NUM)])  # [5]

    c = 0.5 * np.mean(sq)
    xt = np.ascontiguousarray(x.T)  # [D, N] fp32

    # rhs: [128, NJG, 2, JG]  (partition p, col group, k-block, col)
    rhs_host = np.ascontiguousarray(xt.reshape(2, 128, NJG, JG).transpose(1, 2, 0, 3))
    aug_host = np.empty((1, N + 128), np.float32)
    aug_host[0, :N] = (c - 0.5 * sq).astype(np.float32)
    aug_host[0, N:] = 1.0
    scale_f32 = (2.0 * a).astype(np.float32)

    in_maps = []
    for m in range(NCORES):
        r0 = m * ROWS_PER_CORE
        w = xt[:, r0 : r0 + ROWS_PER_CORE]  # [256, 1024]
        w_host = np.ascontiguousarray(w.reshape(2, 128, ROWS_PER_CORE).transpose(1, 0, 2))
        sqm = sq[r0 : r0 + ROWS_PER_CORE]  # [1024]
        # bias[p, iblk*5+b] = -a_b*sq_i - 2*a_b*c  for row i = iblk*128+p
        bias_host = (
            -a[None, None, :] * sqm.reshape(NIBLK, 128).transpose(1, 0)[:, :, None]
            - 2.0 * c * a[None, None, :]
        ).reshape(128, NIBLK * KERNEL_NUM).astype(np.float32)
        bsc_host = np.concatenate(
            [bias_host, np.broadcast_to(scale_f32, (128, KERNEL_NUM))], axis=1
        ).astype(np.float32)
        in_maps.append(
            {
                "xT": rhs_host,
                "wT": w_host,
                "aug2": aug_host,
                "biasscale": bsc_host,
            }
        )
    return in_maps


def _combine(results):
    """results: list of per-core dicts with 'res' [128, NIBLK*NJG*5]."""
    total = 0.0
    for m in range(NCORES):
        r = np.asarray(results[m]["res"], np.float64).reshape(
            128, NIBLK, NJG, KERNEL_NUM
        )
        pos = r[:, :, : NJG // 2, :].sum()  # columns 0..4095  (S block)
        neg = r[:, :, NJG // 2 :, :].sum()  # columns 4096..8191 (T block)
        sign = 1.0 if m < NCORES // 2 else -1.0
        total += sign * (pos - neg)
    return np.float32(total / (B * B))


def kernel(source_features, target_features):
    from concourse.bass_utils import run_bass_kernel_spmd

    nc = _get_program()
    in_maps = _host_prep(source_features, target_features)
    out = run_bass_kernel_spmd(nc, in_maps, list(range(NCORES)))
    return _combine(out.results)
